# revision 1
# baseline (speedup 1.0000x reference)
"""Trainium2 Bass kernel for nn_Deep_Mem_RelativeLocs_ProjectedLowerDim.

out = mem + counts.reshape(IDX_DIMS + (1,1,1)) where counts is an 80000-bin
histogram of hashed rel_vec rows.

Strategy (8 cores, data-parallel over rel_vec rows):
 - Host: transpose rel_vec shard, split into bf16 hi/lo planes (same bytes as
   f32), pack per-super-chunk for efficient DMA.  A -0.5 bias row is folded
   into the hash matmul so that the round-to-nearest f32->i32 cast gives floor.
 - PE: h^T = w.T @ relT via 4 bf16 matmuls (hi*w_hi+mid accum in PSUM[14,:]),
   transpose h back to rows-on-partitions.
 - DVE: h=main+mid, clamp, strides-dot -> flat bucket id; hi=flat//625,
   lo=flat%625; one-hot via is_equal against f16 iotas.
 - PE: counts[hi,lo] += A^T B per 128-row chunk accumulated in PSUM [128,626].
 - ReduceScatter counts across 8 cores; each core adds its 10000-bucket slice
   broadcast over the trailing 200-slab and writes its 8MB output shard.
"""
import numpy as np
import ml_dtypes

# ---- problem constants (hardcoded; must match the harness problem) ----
N_ROWS = 415744
RV_W = 241
N_CORES = 8
ROWS_PER_CORE = N_ROWS // N_CORES            # 51968
CHUNK = 128
N_CHUNKS = ROWS_PER_CORE // CHUNK            # 406
SUP_CHUNKS = 16                              # chunks per super-chunk (DMA unit)
N_SUP = (N_CHUNKS + SUP_CHUNKS - 1) // SUP_CHUNKS   # 26 (last has 6)
IDX_DIMS = (2, 10, 10, 2, 10, 10, 2)
BOUNDS = [d - 1 for d in IDX_DIMS]
STRIDES = [40000, 4000, 400, 200, 20, 2, 1]
N_FLAT = 80000
HI = 128
LO = 625
LOP = 626                                    # padded even width
TRAIL = 200
BPC = N_FLAT // N_CORES                      # 10000 buckets per core
MEM_SIZE = (2, 10, 10, 2, 10, 10, 2, 10, 10, 2)

K0 = 128                                     # feature K-tile sizes
K1 = RV_W - K0                               # 113
SECT = SUP_CHUNKS * CHUNK                    # 2048 rows per super
PL_W = 4 * SECT                              # packed plane width per super

_nc_cache = {}


def _build_nc():
    from contextlib import ExitStack
    import concourse.bacc as bacc
    import concourse.tile as tile
    import concourse.mybir as mybir

    f32 = mybir.dt.float32
    f16 = mybir.dt.float16
    bf16 = mybir.dt.bfloat16
    i32 = mybir.dt.int32
    Alu = mybir.AluOpType

    nc = bacc.Bacc("TRN2", target_bir_lowering=False, debug=False,
                   enable_asserts=False, num_devices=N_CORES)

    planes = nc.dram_tensor("planes", [N_SUP, 128, PL_W], bf16, kind="ExternalInput")
    hwTp = nc.dram_tensor("hwTp", [128, 28], bf16, kind="ExternalInput")
    iota_h = nc.dram_tensor("iota_h", [128, HI], f16, kind="ExternalInput")
    iota_l = nc.dram_tensor("iota_l", [128, LOP], f16, kind="ExternalInput")
    ident = nc.dram_tensor("ident", [128, 16], f32, kind="ExternalInput")
    bounds = nc.dram_tensor("bounds", [128, SUP_CHUNKS * 7], f32, kind="ExternalInput")
    strides = nc.dram_tensor("strides", [128, SUP_CHUNKS * 7], f32, kind="ExternalInput")
    memsh = nc.dram_tensor("memsh", [BPC, TRAIL], f32, kind="ExternalInput")
    out = nc.dram_tensor("out", [BPC, TRAIL], f32, kind="ExternalOutput")

    with tile.TileContext(nc) as tc, ExitStack() as ctx:
        cpool = ctx.enter_context(tc.tile_pool(name="consts", bufs=1))
        relpool = ctx.enter_context(tc.tile_pool(name="rel", bufs=2))
        hsbp = ctx.enter_context(tc.tile_pool(name="hsb", bufs=3))
        hTsbp = ctx.enter_context(tc.tile_pool(name="hTsb", bufs=2))
        arith = ctx.enter_context(tc.tile_pool(name="arith", bufs=2))
        cmpp = ctx.enter_context(tc.tile_pool(name="cmp", bufs=3))
        tailp = ctx.enter_context(tc.tile_pool(name="tail", bufs=1))
        memp = ctx.enter_context(tc.tile_pool(name="mem", bufs=1))
        hps = ctx.enter_context(tc.tile_pool(name="hps", bufs=2, space="PSUM"))
        hTps = ctx.enter_context(tc.tile_pool(name="hTps", bufs=2, space="PSUM"))
        ctps = ctx.enter_context(tc.tile_pool(name="ctps", bufs=1, space="PSUM"))
        dram = ctx.enter_context(tc.tile_pool(name="dram", bufs=1, space="DRAM"))

        # ---- constants
        hwTp_sb = cpool.tile([128, 28], bf16)
        nc.sync.dma_start(hwTp_sb[:], hwTp[:])
        iota_h_sb = cpool.tile([128, HI], f16)
        nc.sync.dma_start(iota_h_sb[:], iota_h[:])
        iota_l_sb = cpool.tile([128, LOP], f16)
        nc.sync.dma_start(iota_l_sb[:], iota_l[:])
        id_sb = cpool.tile([128, 16], f32)
        nc.sync.dma_start(id_sb[:], ident[:])
        bounds_sb = cpool.tile([128, SUP_CHUNKS * 7], f32)
        nc.sync.dma_start(bounds_sb[:], bounds[:])
        strides_sb = cpool.tile([128, SUP_CHUNKS * 7], f32)
        nc.sync.dma_start(strides_sb[:], strides[:])

        mem_sb = memp.tile([125, 16000], f32)
        memr = memsh[:].rearrange("(p b) t -> p (b t)", p=125)

        counts_ps = ctps.tile([128, LOP], f32)

        chunk_idx = 0
        for s in range(N_SUP):
            S = min(SUP_CHUNKS, N_CHUNKS - s * SUP_CHUNKS)
            pl = relpool.tile([128, PL_W], bf16, tag="pl")
            nc.sync.dma_start(pl[:], planes[s, :, :])
            # sections within pl: 0:hi_k0 1:lo_k0 2:hi_k1 3:lo_k1
            hi_k0 = pl[:, 0 * SECT:0 * SECT + S * CHUNK]
            lo_k0 = pl[:, 1 * SECT:1 * SECT + S * CHUNK]
            hi_k1 = pl[0:K1 + 1, 2 * SECT:2 * SECT + S * CHUNK]   # +1: ones bias row
            lo_k1 = pl[0:K1, 3 * SECT:3 * SECT + S * CHUNK]

            # mem prefetch spread over mid supers (4 x 2MB)
            if 18 <= s <= 21:
                q = s - 18
                nc.sync.dma_start(mem_sb[:, q * 4000:(q + 1) * 4000],
                                  memr[:, q * 4000:(q + 1) * 4000])

            hT_ps = hTps.tile([128, SUP_CHUNKS * 14], f32, tag="hTps")
            for g in range(S // 2):
                cols = slice(g * 256, (g + 1) * 256)
                h_ps = hps.tile([14, 256], f32, tag="hps")
                nc.tensor.matmul(h_ps[:], hwTp_sb[:, 0:14], hi_k0[:, cols],
                                 start=True, stop=False)
                nc.tensor.matmul(h_ps[:], hwTp_sb[0:K1 + 1, 14:28], hi_k1[:, cols],
                                 start=False, stop=False)
                nc.tensor.matmul(h_ps[0:7, :], hwTp_sb[:, 0:7], lo_k0[:, cols],
                                 start=False, stop=False)
                nc.tensor.matmul(h_ps[0:7, :], hwTp_sb[0:K1, 14:21], lo_k1[:, cols],
                                 start=False, stop=True)
                h_sb = hsbp.tile([14, 256], f32, tag="hsb")
                nc.scalar.copy(h_sb[:], h_ps[:])
                for j in range(2):
                    cc = 2 * g + j
                    nc.tensor.transpose(hT_ps[:, cc * 14:(cc + 1) * 14],
                                        h_sb[:, j * 128:(j + 1) * 128],
                                        id_sb[0:14, 0:14])

            hT_sb = hTsbp.tile([128, SUP_CHUNKS * 14], f32, tag="hTsb")
            nc.scalar.copy(hT_sb[:, 0:S * 14], hT_ps[:, 0:S * 14])

            # DVE arithmetic (bias -0.5 already folded into h)
            hfloor = arith.tile([128, SUP_CHUNKS * 7], i32, tag="hfloor")
            main_ap = hT_sb[:, 0:S * 14].rearrange("p (c t) -> p c t", t=14)[:, :, 0:7]
            mid_ap = hT_sb[:, 0:S * 14].rearrange("p (c t) -> p c t", t=14)[:, :, 7:14]
            hf_ap = hfloor[:, 0:S * 7].rearrange("p (c t) -> p c t", t=7)
            nc.vector.tensor_tensor(hf_ap, main_ap, mid_ap, Alu.add)

            clamped = arith.tile([128, SUP_CHUNKS * 7], f32, tag="clamped")
            nc.vector.tensor_tensor(clamped[:, 0:S * 7], hfloor[:, 0:S * 7],
                                    bounds_sb[:, 0:S * 7], Alu.min)
            nc.vector.tensor_tensor(clamped[:, 0:S * 7], clamped[:, 0:S * 7],
                                    strides_sb[:, 0:S * 7], Alu.mult)
            flat = arith.tile([128, SUP_CHUNKS], f32, tag="flat")
            nc.vector.tensor_reduce(
                flat[:, 0:S],
                clamped[:, 0:S * 7].rearrange("p (c t) -> p c t", t=7),
                mybir.AxisListType.X, Alu.add)

            hi_i = arith.tile([128, SUP_CHUNKS], i32, tag="hi_i")
            nc.vector.tensor_scalar(hi_i[:, 0:S], flat[:, 0:S], 1.0 / 625.0, -0.5,
                                    Alu.mult, Alu.add)
            hi_f = arith.tile([128, SUP_CHUNKS], f32, tag="hi_f")
            nc.vector.tensor_copy(hi_f[:, 0:S], hi_i[:, 0:S])
            neg = arith.tile([128, SUP_CHUNKS], f32, tag="neg")
            nc.vector.tensor_scalar(neg[:, 0:S], hi_i[:, 0:S], -625.0, None, Alu.mult)
            lo_f = arith.tile([128, SUP_CHUNKS], f32, tag="lo_f")
            nc.vector.tensor_tensor(lo_f[:, 0:S], neg[:, 0:S], flat[:, 0:S], Alu.add)

            for j in range(S):
                A = cmpp.tile([128, HI], bf16, tag="A")
                nc.vector.tensor_scalar(A[:], iota_h_sb[:], hi_f[:, j:j + 1], None,
                                        Alu.is_equal)
                B = cmpp.tile([128, LOP], bf16, tag="B")
                nc.vector.tensor_scalar(B[:], iota_l_sb[:], lo_f[:, j:j + 1], None,
                                        Alu.is_equal)
                first = chunk_idx == 0
                last = chunk_idx == N_CHUNKS - 1
                nc.tensor.matmul(counts_ps[:, 0:512], A[:], B[:, 0:512],
                                 start=first, stop=last, skip_group_check=True)
                nc.tensor.matmul(counts_ps[:, 512:LOP], A[:], B[:, 512:LOP],
                                 start=first, stop=last, skip_group_check=True)
                chunk_idx += 1

        # ---- tail: reduce counts across cores, add to mem shard, write out
        counts_sb = tailp.tile([128, LOP], f32)
        nc.vector.tensor_copy(counts_sb[:], counts_ps[:])

        counts_dram = dram.tile([128, LO], f32)
        red_dram = dram.tile([16, LO], f32)
        nc.sync.dma_start(counts_dram[:], counts_sb[:, 0:LO])
        nc.gpsimd.collective_compute(
            "ReduceScatter", Alu.add,
            replica_groups=[list(range(N_CORES))],
            ins=[counts_dram.opt()],
            outs=[red_dram.opt()],
        )
        red_sb = tailp.tile([125, 80], f32)
        nc.sync.dma_start(red_sb[:], red_dram[:].rearrange("a b -> (a b)").rearrange("(p c) -> p c", p=125))

        red_b = red_sb[:].unsqueeze(2).broadcast_to([125, 80, TRAIL])
        mem3 = mem_sb[:].rearrange("p (c t) -> p c t", t=TRAIL)
        nc.vector.tensor_tensor(mem3, mem3, red_b, Alu.add)

        outr = out[:].rearrange("(p b) t -> p (b t)", p=125)
        for q in range(4):
            nc.sync.dma_start(outr[:, q * 4000:(q + 1) * 4000],
                              mem_sb[:, q * 4000:(q + 1) * 4000])

    nc.compile()
    return nc


def _host_prep(rel_vec, hash_w):
    """Build per-core packed bf16 hi/lo planes + constant tensors."""
    bf = ml_dtypes.bfloat16
    consts = {}
    w = hash_w.T.astype(np.float32)                      # [241, 7]
    w_hi = w.astype(bf).astype(np.float32)
    w_mid = (w - w_hi).astype(bf).astype(np.float32)
    hwTp = np.zeros((128, 28), np.float32)
    hwTp[:, 0:7] = w_hi[0:K0]
    hwTp[:, 7:14] = w_mid[0:K0]
    hwTp[0:K1, 14:21] = w_hi[K0:RV_W]
    hwTp[0:K1, 21:28] = w_mid[K0:RV_W]
    hwTp[K1, 14:21] = -0.5                               # floor bias row
    consts["hwTp"] = hwTp.astype(bf)

    consts["iota_h"] = np.broadcast_to(
        np.arange(HI, dtype=np.float16)[None, :], (128, HI)).copy()
    il = np.arange(LOP, dtype=np.float16)
    il[LO] = 10000.0                                     # pad col never matches
    consts["iota_l"] = np.broadcast_to(il[None, :], (128, LOP)).copy()
    ident = np.zeros((128, 16), np.float32)
    ident[0:14, 0:14] = np.eye(14, dtype=np.float32)
    consts["ident"] = ident
    consts["bounds"] = np.broadcast_to(
        np.tile(np.array(BOUNDS, np.float32), SUP_CHUNKS)[None, :],
        (128, SUP_CHUNKS * 7)).copy()
    consts["strides"] = np.broadcast_to(
        np.tile(np.array(STRIDES, np.float32), SUP_CHUNKS)[None, :],
        (128, SUP_CHUNKS * 7)).copy()

    # per-core planes
    pad_rows = N_SUP * SECT - ROWS_PER_CORE              # 1280
    planes_all = []
    for c in range(N_CORES):
        shard = rel_vec[c * ROWS_PER_CORE:(c + 1) * ROWS_PER_CORE]
        if pad_rows:
            shard = np.concatenate(
                [shard, np.zeros((pad_rows, RV_W), np.float32)], axis=0)
        R = shard.reshape(N_SUP, SECT, RV_W)
        hi = R.astype(bf)
        lo = (R - hi.astype(np.float32)).astype(bf)
        hiT = np.ascontiguousarray(hi.transpose(0, 2, 1))    # [S, 241, 2048]
        loT = np.ascontiguousarray(lo.transpose(0, 2, 1))
        pk = np.zeros((N_SUP, 128, PL_W), bf)
        pk[:, :, 0 * SECT:1 * SECT] = hiT[:, 0:K0]
        pk[:, :, 1 * SECT:2 * SECT] = loT[:, 0:K0]
        pk[:, 0:K1, 2 * SECT:3 * SECT] = hiT[:, K0:RV_W]
        pk[:, K1, 2 * SECT:3 * SECT] = bf(1.0)               # ones bias row
        pk[:, 0:K1, 3 * SECT:4 * SECT] = loT[:, K0:RV_W]
        planes_all.append(pk)
    return consts, planes_all


def kernel(rel_vec, hash_w, mem):
    from concourse import bass_utils

    rel_vec = np.asarray(rel_vec, np.float32)
    hash_w = np.asarray(hash_w, np.float32)
    mem = np.asarray(mem, np.float32)

    if "nc" not in _nc_cache:
        _nc_cache["nc"] = _build_nc()
    nc = _nc_cache["nc"]

    consts, planes_all = _host_prep(rel_vec, hash_w)
    mem_flat = mem.reshape(N_FLAT, TRAIL)

    in_maps = []
    for c in range(N_CORES):
        m = dict(consts)
        m["planes"] = planes_all[c]
        m["memsh"] = np.ascontiguousarray(mem_flat[c * BPC:(c + 1) * BPC])
        in_maps.append(m)

    res = bass_utils.run_bass_kernel_spmd(nc, in_maps, core_ids=list(range(N_CORES)))
    out = np.concatenate([r["out"] for r in res.results], axis=0)
    return out.reshape(MEM_SIZE)



# revision 2
# speedup vs baseline: 1.3442x; 1.3442x over previous
"""Trainium2 Bass kernel for nn_Deep_Mem_RelativeLocs_ProjectedLowerDim (v2).

out = mem + counts.reshape(IDX_DIMS + (1,1,1)) where counts is an 80000-bin
histogram of hashed rel_vec rows.

Strategy (8 cores, data-parallel over rel_vec rows):
 - Host: single f16 transposed plane per super-chunk (2 K-tiles side by side,
   ones bias row folded in so PSUM holds h-0.5 and round-to-nearest = floor).
 - PE hash: rows-as-stationary matmuls — lhsT = plane chunk [K,128 rows],
   rhs = w^T [K,7] — h lands directly rows-on-partitions in PSUM, no
   transposes or PSUM->SBUF copies.
 - Pool(gpsimd): clamp/stride-dot/reduce to flat bucket ids + ~half the
   128-wide hi one-hots. ACT: other hi one-hots via Square(x-hi)->Relu(1-y)
   (both funcs live in every act table set -> no table reloads).
 - DVE: hi/lo extraction + all 626-wide lo one-hots in f16 (2x mode).
 - PE counts: fp8e5 DoubleRow matmuls (2 chunks per matmul, 0.5 cyc/row)
   reading the f16 one-hots' high bytes via bitcast (f16 1.0 high byte =
   e5m2 1.0 exactly; 0.0 = 0x00) — 4x cheaper than bf16, bit-exact.
 - Tail: one ReduceScatter of the 80000-bin counts, then mem add + output
   store pipelined in 8 slices.
"""
import numpy as np

# ---- problem constants (hardcoded; must match the harness problem) ----
N_ROWS = 415744
RV_W = 241
N_CORES = 8
ROWS_PER_CORE = N_ROWS // N_CORES            # 51968
CHUNK = 128
N_CHUNKS = ROWS_PER_CORE // CHUNK            # 406
N_PAIRS = N_CHUNKS // 2                      # 203
SUP_CHUNKS = 16                              # chunks per super-chunk (DMA unit)
N_SUP = (N_CHUNKS + SUP_CHUNKS - 1) // SUP_CHUNKS   # 26 (last has 6)
IDX_DIMS = (2, 10, 10, 2, 10, 10, 2)
BOUNDS = [d - 1 for d in IDX_DIMS]
STRIDES = [40000, 4000, 400, 200, 20, 2, 1]
N_FLAT = 80000
HI = 128
LO = 625
LOP = 626                                    # padded even width
TRAIL = 200
BPC = N_FLAT // N_CORES                      # 10000 buckets per core
MEM_SIZE = (2, 10, 10, 2, 10, 10, 2, 10, 10, 2)

K0 = 128                                     # feature K-tile sizes
K1 = RV_W - K0                               # 113 (+1 ones bias row -> 114)
SECT = SUP_CHUNKS * CHUNK                    # 2048 rows per super
PL_W = 2 * SECT                              # packed plane width per super
# super sizes in chunks: small first super -> short prologue critical path
SUPER_SIZES = [16] * 25 + [6]                # sums to 406
assert sum(SUPER_SIZES) == N_CHUNKS
SUPER_BASE = [sum(SUPER_SIZES[:i]) for i in range(len(SUPER_SIZES))]
N_SUPERS = len(SUPER_SIZES)                  # 26
PAIR_SPLIT = 165                             # pairs [0,165) -> RS_a mid-loop
POOL_PAIRS = 5                               # pairs 0..4 of each super on Pool
# Engine feature config (validated on HW): Pool runs tensor_tensor ops and
# the A2 is_equal one-hots; the small mixed-i32 copy/scalar ops fault the
# GPSIMD ucode and stay on DVE.
USE_GPSIMD_TT = True   # Pool tensor_tensor (mult/add)
USE_GPSIMD_TS = False  # Pool small tensor_scalar/copy (faults Q7 ucode)
USE_GPSIMD_A2 = True   # Pool A2 is_equal
USE_DR = True          # fp8e5 DoubleRow count matmuls via f16-high-byte bitcast
USE_ACT = True         # ACT one-hot path (Square -> wide Relu)

_nc_cache = {}


def _build_nc():
    from contextlib import ExitStack
    import concourse.bacc as bacc
    import concourse.tile as tile
    import concourse.mybir as mybir

    f32 = mybir.dt.float32
    f16 = mybir.dt.float16
    f8e5 = mybir.dt.float8e5
    i32 = mybir.dt.int32
    Alu = mybir.AluOpType
    Act = mybir.ActivationFunctionType
    DR = mybir.MatmulPerfMode.DoubleRow

    nc = bacc.Bacc("TRN2", target_bir_lowering=False, debug=False,
                   enable_asserts=False, num_devices=N_CORES)

    planes = nc.dram_tensor("planes", [N_SUP, 128, PL_W], f16, kind="ExternalInput")
    hwT = nc.dram_tensor("hwT", [128, 14], f16, kind="ExternalInput")
    iota_h = nc.dram_tensor("iota_h", [128, HI], f16, kind="ExternalInput")
    iota_hn = nc.dram_tensor("iota_hn", [128, HI], f16, kind="ExternalInput")
    iota_l = nc.dram_tensor("iota_l", [128, LOP], f16, kind="ExternalInput")
    bounds = nc.dram_tensor("bounds", [128, SUP_CHUNKS * 7], f32, kind="ExternalInput")
    strides = nc.dram_tensor("strides", [128, SUP_CHUNKS * 7], f32, kind="ExternalInput")
    memsh = nc.dram_tensor("memsh", [BPC, TRAIL], f32, kind="ExternalInput")
    out = nc.dram_tensor("out", [BPC, TRAIL], f32, kind="ExternalOutput")

    with tile.TileContext(nc) as tc, ExitStack() as ctx:
        cpool = ctx.enter_context(tc.tile_pool(name="consts", bufs=1))
        relpool = ctx.enter_context(tc.tile_pool(name="rel", bufs=3))
        arith = ctx.enter_context(tc.tile_pool(name="arith", bufs=2))
        apool = ctx.enter_context(tc.tile_pool(name="apool", bufs=6))
        bpool = ctx.enter_context(tc.tile_pool(name="bpool", bufs=6))
        tpool = ctx.enter_context(tc.tile_pool(name="tpool", bufs=2))
        tailp = ctx.enter_context(tc.tile_pool(name="tail", bufs=1))
        memp = ctx.enter_context(tc.tile_pool(name="mem", bufs=1))
        hps = ctx.enter_context(tc.tile_pool(name="hps", bufs=2, space="PSUM"))
        ctps = ctx.enter_context(tc.tile_pool(name="ctps", bufs=1, space="PSUM"))
        dram = ctx.enter_context(tc.tile_pool(name="dram", bufs=1, space="DRAM"))

        mem_sb = memp.tile([125, 16000], f32)
        memr = memsh[:].rearrange("(p b) t -> p (b t)", p=125)

        counts_ps = ctps.tile([128, LOP], f32)

        def n_chunks(s):
            return SUPER_SIZES[s]

        pl_tiles = {}

        def dma_plane(s):
            S = n_chunks(s)
            pl = relpool.tile([128, PL_W], f16, tag="pl")
            nc.sync.dma_start(pl[:, 0:2 * S * CHUNK], planes[s, :, 0:2 * S * CHUNK])
            pl_tiles[s] = pl

        def issue_hash(s):
            """PE: per-chunk rows-as-stationary hash matmuls -> hT_ps[s]."""
            S = n_chunks(s)
            pl = pl_tiles.pop(s)
            hT_ps = hps.tile([128, SUP_CHUNKS * 7], f32, tag="hT")
            for c in range(S):
                nc.tensor.matmul(hT_ps[:, c * 7:(c + 1) * 7],
                                 pl[:, c * CHUNK:(c + 1) * CHUNK],
                                 hwT_sb[:, 0:7],
                                 start=True, stop=False, skip_group_check=True)
                nc.tensor.matmul(hT_ps[:, c * 7:(c + 1) * 7],
                                 pl[0:K1 + 1, S * CHUNK + c * CHUNK:
                                    S * CHUNK + (c + 1) * CHUNK],
                                 hwT_sb[0:K1 + 1, 7:14],
                                 start=False, stop=True, skip_group_check=True)
            return hT_ps

        def issue_pool_misc(s, hT_ps):
            """ACT: floor via PSUM f32 -> i32 RNE copy of h-0.5 (GPSIMD can't
            read PSUM); DVE: clamp to bounds (f32 out); Pool: f32 stride
            mult (Pool supports float ops only)."""
            S = n_chunks(s)
            hT_i = arith.tile([128, SUP_CHUNKS * 7], i32, tag="hT_i")
            nc.scalar.copy(hT_i[:, 0:S * 7], hT_ps[:, 0:S * 7])
            hfloor = arith.tile([128, SUP_CHUNKS * 7], f32, tag="hfloor")
            nc.vector.tensor_tensor(hfloor[:, 0:S * 7], hT_i[:, 0:S * 7],
                                    bounds_sb[:, 0:S * 7], Alu.min)
            flm = arith.tile([128, SUP_CHUNKS * 7], f32, tag="flm")
            eng = nc.gpsimd if USE_GPSIMD_TT else nc.vector
            eng.tensor_tensor(flm[:, 0:S * 7], hfloor[:, 0:S * 7],
                              strides_sb[:, 0:S * 7], Alu.mult)
            return flm

        def issue_flat(s, flm):
            """DVE: reduce stride-dot products to flat bucket ids."""
            S = n_chunks(s)
            flat = arith.tile([128, SUP_CHUNKS], f32, tag="flat")
            nc.vector.tensor_reduce(
                flat[:, 0:S],
                flm[:, 0:S * 7].rearrange("p (c t) -> p c t", t=7),
                mybir.AxisListType.X, Alu.add)
            return flat

        def issue_dve_misc(s, flm):
            """DVE: flat reduce + floor-divide; Pool: the rest of hi/lo."""
            S = n_chunks(s)
            flat = issue_flat(s, flm)
            hi_neg = arith.tile([128, SUP_CHUNKS], i32, tag="hi_neg")
            nc.vector.tensor_scalar(hi_neg[:, 0:S], flat[:, 0:S],
                                    -1.0 / 625.0, 0.5 - 1.0 / 1250.0,
                                    Alu.mult, Alu.add)
            eng_s = nc.gpsimd if USE_GPSIMD_TS else nc.vector
            eng_t = nc.gpsimd if USE_GPSIMD_TT else nc.vector
            hi_negf = arith.tile([128, SUP_CHUNKS], f32, tag="hi_negf")
            eng_s.tensor_copy(hi_negf[:, 0:S], hi_neg[:, 0:S])
            lo_t = arith.tile([128, SUP_CHUNKS], f32, tag="lo_t")
            eng_s.tensor_scalar(lo_t[:, 0:S], hi_neg[:, 0:S], 625.0, None,
                                Alu.mult)
            lo_f = arith.tile([128, SUP_CHUNKS], f32, tag="lo_f")
            eng_t.tensor_tensor(lo_f[:, 0:S], lo_t[:, 0:S], flat[:, 0:S],
                                Alu.add)
            return hi_negf, lo_f

        # ---- prologue: plane 0 first (critical path), then consts, plane 1
        dma_plane(0)
        hwT_sb = cpool.tile([128, 14], f16)
        nc.sync.dma_start(hwT_sb[:], hwT[:])
        iota_h_sb = cpool.tile([128, HI], f16)
        nc.sync.dma_start(iota_h_sb[:], iota_h[:])
        iota_hn_sb = cpool.tile([128, HI], f16)
        nc.sync.dma_start(iota_hn_sb[:], iota_hn[:])
        iota_l_sb = cpool.tile([128, LOP], f16)
        nc.sync.dma_start(iota_l_sb[:], iota_l[:])
        bounds_sb = cpool.tile([128, SUP_CHUNKS * 7], f32)
        nc.sync.dma_start(bounds_sb[:], bounds[:])
        strides_sb = cpool.tile([128, SUP_CHUNKS * 7], f32)
        nc.sync.dma_start(strides_sb[:], strides[:])
        dma_plane(1)
        hT_cur = issue_hash(0)
        flm_cur = issue_pool_misc(0, hT_cur)
        misc_cur = issue_dve_misc(0, flm_cur)

        chunk_pair = 0
        for s in range(N_SUPERS):
            S = n_chunks(s)
            if s + 2 < N_SUPERS:
                dma_plane(s + 2)
            # mem prefetch spread over mid supers (4 x 2MB)
            if s in (16, 18, 20, 22):
                q = (s - 16) // 2
                nc.sync.dma_start(mem_sb[:, q * 4000:(q + 1) * 4000],
                                  memr[:, q * 4000:(q + 1) * 4000])
            hT_next = issue_hash(s + 1) if s + 1 < N_SUPERS else None
            hi_negf, lo_f = misc_cur

            flm_next = None
            misc_next = None
            for p in range(S // 2):
                j0, j1 = 2 * p, 2 * p + 1
                A2 = apool.tile([128, 2 * HI], f16, tag="A2")
                if p < POOL_PAIRS and USE_GPSIMD_A2:
                    for ji, j in enumerate((j0, j1)):
                        nc.gpsimd.tensor_scalar(A2[:, ji * HI:(ji + 1) * HI],
                                                iota_hn_sb[:],
                                                hi_negf[:, j:j + 1], None,
                                                Alu.is_equal)
                elif not USE_ACT:
                    for ji, j in enumerate((j0, j1)):
                        nc.vector.tensor_scalar(A2[:, ji * HI:(ji + 1) * HI],
                                                iota_hn_sb[:],
                                                hi_negf[:, j:j + 1], None,
                                                Alu.is_equal)
                else:
                    # ACT path: y=(iota-hi)^2 per half, then one wide
                    # Relu(1-y) over both halves (Square+Relu share every
                    # act table set -> no table reloads)
                    t = tpool.tile([128, 2 * HI], f16, tag="t")
                    for ji, j in enumerate((j0, j1)):
                        nc.scalar.activation(t[:, ji * HI:(ji + 1) * HI],
                                             iota_h_sb[:], Act.Square,
                                             bias=hi_negf[:, j:j + 1], scale=1.0)
                    nc.scalar.activation(A2[:], t[:], Act.Relu,
                                         bias=1.0, scale=-1.0)
                B2 = bpool.tile([128, 2 * LOP], f16, tag="B2")
                nc.vector.tensor_scalar(B2[:, 0:LOP], iota_l_sb[:],
                                        lo_f[:, j0:j0 + 1], None, Alu.is_equal)
                nc.vector.tensor_scalar(B2[:, LOP:2 * LOP], iota_l_sb[:],
                                        lo_f[:, j1:j1 + 1], None, Alu.is_equal)
                cps = counts_ps
                first = chunk_pair == 0
                last = chunk_pair == N_PAIRS - 1
                # counts span two 2KB PSUM zero-regions: [0:512) cols and
                # [512:626). start=True only on the first matmul touching each
                # region; the region-wide pending-zero mark makes the other
                # first-touch writes replace (not accumulate) as required.
                if USE_DR:
                    a3 = A2[:].bitcast(f8e5)[:, 1::2].rearrange(
                        "p (two m) -> p two m", two=2)
                    b3 = B2[:].bitcast(f8e5)[:, 1::2].rearrange(
                        "p (two n) -> p two n", two=2)
                    nc.tensor.matmul(cps[:, 0:256], a3, b3[:, :, 0:256],
                                     start=first, stop=last, perf_mode=DR,
                                     skip_group_check=True)
                    nc.tensor.matmul(cps[:, 256:512], a3, b3[:, :, 256:512],
                                     start=False, stop=last, perf_mode=DR,
                                     skip_group_check=True)
                    nc.tensor.matmul(cps[:, 512:LOP], a3, b3[:, :, 512:LOP],
                                     start=first, stop=last, perf_mode=DR,
                                     skip_group_check=True)
                else:
                    for ji in range(2):
                        fa = first and ji == 0
                        la = last and ji == 1
                        Ah = A2[:, ji * HI:(ji + 1) * HI]
                        Bh = B2[:, ji * LOP:(ji + 1) * LOP]
                        nc.tensor.matmul(cps[:, 0:512], Ah, Bh[:, 0:512],
                                         start=fa, stop=la,
                                         skip_group_check=True)
                        nc.tensor.matmul(cps[:, 512:LOP], Ah, Bh[:, 512:LOP],
                                         start=fa, stop=la,
                                         skip_group_check=True)
                chunk_pair += 1
                # issue next super's Pool misc after a couple of pairs so Pool
                # doesn't stall on hT(s+1), and DVE misc a few pairs later
                if p == 0 and hT_next is not None:
                    flm_next = issue_pool_misc(s + 1, hT_next)
                if p == min(3, S // 2 - 1) and flm_next is not None \
                        and misc_next is None:
                    misc_next = issue_dve_misc(s + 1, flm_next)
            if hT_next is not None and flm_next is None:
                # short super fallback (shouldn't happen: only last is short)
                flm_next = issue_pool_misc(s + 1, hT_next)
            if flm_next is not None and misc_next is None:
                misc_next = issue_dve_misc(s + 1, flm_next)
            misc_cur = misc_next

        # ---- tail: reduce counts across cores, add to mem shard, write out
        counts_sb = tailp.tile([128, LO], f32)
        nc.vector.tensor_copy(counts_sb[:], counts_ps[:, 0:LO])
        counts_dram = dram.tile([128, LO], f32)
        red_dram = dram.tile([16, LO], f32)
        nc.sync.dma_start(counts_dram[:], counts_sb[:])
        nc.gpsimd.collective_compute(
            "ReduceScatter", Alu.add,
            replica_groups=[list(range(N_CORES))],
            ins=[counts_dram.opt()],
            outs=[red_dram.opt()],
        )
        red_sb = tailp.tile([125, 80], f32)
        nc.sync.dma_start(red_sb[:], red_dram[:].rearrange("a b -> (a b)")
                          .rearrange("(p c) -> p c", p=125))

        outr = out[:].rearrange("(p b) t -> p (b t)", p=125)
        for q in range(8):
            mem3 = mem_sb[:, q * 2000:(q + 1) * 2000].rearrange(
                "p (b t) -> p b t", t=TRAIL)
            red_b = red_sb[:, q * 10:(q + 1) * 10].unsqueeze(2).broadcast_to(
                [125, 10, TRAIL])
            nc.vector.tensor_tensor(mem3, mem3, red_b, Alu.add)
            nc.sync.dma_start(outr[:, q * 2000:(q + 1) * 2000],
                              mem_sb[:, q * 2000:(q + 1) * 2000])

    nc.compile()
    return nc


def _host_prep(rel_vec, hash_w):
    """Build per-core packed f16 transposed planes + constant tensors."""
    f16 = np.float16
    consts = {}
    w = hash_w.T.astype(np.float32)                      # [241, 7]
    hwT = np.zeros((128, 14), np.float32)
    hwT[:, 0:7] = w[0:K0]
    hwT[0:K1, 7:14] = w[K0:RV_W]
    hwT[K1, 7:14] = -0.5                                 # floor bias row
    consts["hwT"] = hwT.astype(f16)

    consts["iota_h"] = np.broadcast_to(
        np.arange(HI, dtype=f16)[None, :], (128, HI)).copy()
    consts["iota_hn"] = np.broadcast_to(
        -np.arange(HI, dtype=f16)[None, :], (128, HI)).copy()
    il = np.arange(LOP, dtype=f16)
    il[LO] = 30000.0                                     # pad col never matches
    consts["iota_l"] = np.broadcast_to(il[None, :], (128, LOP)).copy()
    consts["bounds"] = np.broadcast_to(
        np.tile(np.array(BOUNDS, np.float32), SUP_CHUNKS)[None, :],
        (128, SUP_CHUNKS * 7)).copy()
    consts["strides"] = np.broadcast_to(
        np.tile(np.array(STRIDES, np.float32), SUP_CHUNKS)[None, :],
        (128, SUP_CHUNKS * 7)).copy()

    planes_all = []
    for c in range(N_CORES):
        shard = rel_vec[c * ROWS_PER_CORE:(c + 1) * ROWS_PER_CORE].astype(f16)
        pk = np.zeros((N_SUPERS, 128, PL_W), f16)
        for s in range(N_SUPERS):
            S = SUPER_SIZES[s]
            base = SUPER_BASE[s] * CHUNK
            Rt = shard[base:base + S * CHUNK].T          # [241, S*128]
            pk[s, :, 0:S * CHUNK] = Rt[0:K0]
            pk[s, 0:K1, S * CHUNK:2 * S * CHUNK] = Rt[K0:RV_W]
            pk[s, K1, S * CHUNK:2 * S * CHUNK] = f16(1.0)   # ones bias row
        planes_all.append(pk)
    return consts, planes_all


def kernel(rel_vec, hash_w, mem):
    from concourse import bass_utils

    rel_vec = np.asarray(rel_vec, np.float32)
    hash_w = np.asarray(hash_w, np.float32)
    mem = np.asarray(mem, np.float32)

    if "nc" not in _nc_cache:
        _nc_cache["nc"] = _build_nc()
    nc = _nc_cache["nc"]

    consts, planes_all = _host_prep(rel_vec, hash_w)
    mem_flat = mem.reshape(N_FLAT, TRAIL)

    in_maps = []
    for c in range(N_CORES):
        m = dict(consts)
        m["planes"] = planes_all[c]
        m["memsh"] = np.ascontiguousarray(mem_flat[c * BPC:(c + 1) * BPC])
        in_maps.append(m)

    res = bass_utils.run_bass_kernel_spmd(nc, in_maps, core_ids=list(range(N_CORES)))
    out = np.concatenate([r["out"] for r in res.results], axis=0)
    return out.reshape(MEM_SIZE)
